# revision 51
# baseline (speedup 1.0000x reference)
"""Trainium2 Bass kernel for nn_DTHyperNet (soft decision tree hypernetwork).

Contract: kernel(**inputs) takes the FULL unsharded inputs (B=8192) as
numpy arrays and returns the FULL [8192, 100] float32 output. Internally
the batch is sharded 8 ways (pure data parallel, weights replicated) and
one Bass/Tile program is compiled and run SPMD on NeuronCores 0-7.

Math (eval mode):
  trunk:  h = relu(bn(x @ w_in + b_in))  [+ residual gelu blocks, which
          collapse to identity when bn2 weight/bias are zero - detected
          from the actual input values and skipped]
  heads:  fi/fs = h @ w_fi/w_fs  (15 nodes x 512 features)
          lnc   = h @ w_lc       (16 leaves x 100 classes)
  per node: sd = sigmoid(sum_f softmax(fi)*(x - fs))
  routing coeff[leaf] = prod_d (sd or 1-sd) along the tree path
  out = sum_l coeff_l * lnc_l

v2 implementation (PE-bound at the fp8 hardware roofline, ~193us vs the
284us fp16 v1; engine busy: PE ~165us, ACT ~127, DVE ~95, Pool ~70):
  - fi/fs matmuls run in fp8 e4m3 with perf_mode=DoubleRow (2 fp8
    weights per PE cell, 256-deep contraction per instruction): measured
    217ns per 512-col matmul warm = 155 TF/s, the fp8 peak. Weights are
    host-scaled by 16 (undone via the ACT exp scale and the DVE dot
    scale) so the fp8 normal range is well used; fi/fs rounding errors
    are attenuated ~100x through the softmax/sigmoid, total rel err
    ~9e-4 (lnc in fp8 was measured at 4e-2 - over the gate - so lnc and
    the trunk stay fp16).
  - The x term of the soft-decision numerator is folded into the fs
    PSUM with a 16*I identity f16 matmul per (node, tile), so ONE DVE
    dot (x-fs)*P per (node, tile) covers the numerator; den rides the
    ACT exp accumulator for free. (All accumulating DVE ops run at 1x
    regardless of dtype, and GPSIMD can neither touch PSUM nor
    accumulate, so the PE-heavy split pipelines best: a PE-paced
    schedule shows only ~3% bubbles vs ~37% when DVE paces.)
  - fp8 makes both head weight matrices small enough (~61KB/partition)
    to stay resident in SBUF, enabling a tile-major single pass: each
    batch tile runs all 15 nodes then finalizes (sigmoid + routing +
    weighted leaf sum) on Pool under the next tile's node loop; only
    the last tile's chain is exposed, run on DVE+Pool class-halves.
  - ~4us of dummy accumulation-group matmuls warm the PE p-state
    (0.65->2.4GHz) while the trunk DMAs land; dma_start issue costs
    ~600ns each on the Sync engine, so loads are few and coarse with
    the trunk-critical ones first.
  - This walrus build rejects instructions with more than one semaphore
    wait, so a post-pass splits multi-wait instructions by hoisting
    excess waits onto same-engine NOPs.
"""
import os
import sys
import types
import numpy as np
import ml_dtypes
from contextlib import ExitStack


def _install_axon_ntff_hook():
    """Expose the axon NTFF profiling hook under antenv.axon_hooks so
    run_bass_kernel_spmd(trace=True) works in this container. Harmless
    no-op when the hook or .so is unavailable."""
    if 'antenv.axon_hooks' in sys.modules:
        return
    try:
        import antenv
    except ImportError:
        return
    hook = None
    try:
        from trn_agent_boot.trn_boot import _ntff_profile_via_ctypes
        hook = _ntff_profile_via_ctypes('/opt/axon/libaxon_pjrt.so')
    except Exception:
        hook = None
    mod = types.ModuleType('antenv.axon_hooks')
    mod._hook = hook
    mod.get_axon_ntff_profile_hook = lambda: mod._hook
    mod.set_axon_ntff_profile_hook = lambda h: setattr(mod, '_hook', h)
    antenv.axon_hooks = mod
    sys.modules['antenv.axon_hooks'] = mod


_install_axon_ntff_hook()

import concourse.bass as bass
import concourse.tile as tile
import concourse.mybir as mybir
import bass_rust as _br
from concourse import bass_utils


def fix_sync_waits(nc, max_waits=1):
    """Split instructions with >max_waits sem waits: excess waits move to
    preceding same-engine InstNoOp instructions (this walrus build rejects
    multi-wait instructions)."""
    n_split = 0
    uid = 0
    for f in nc.m.functions:
        for bb in f.blocks:
            newl = []
            dirty = False
            for inst in bb.instructions:
                si = inst.sync_info
                if si is not None:
                    waits = list(si.on_wait or [])
                    if len(waits) > max_waits:
                        n_split += 1
                        dirty = True
                        excess = waits[:-max_waits]
                        keep = waits[-max_waits:]
                        for i in range(0, len(excess), max_waits):
                            nop = mybir.InstNoOp(name=f"waitnop{uid}", ins=[], outs=[])
                            uid += 1
                            nop.engine = inst.engine
                            nop.sync_info = _br.SyncInfo(
                                on_wait=excess[i:i+max_waits], on_update=[])
                            newl.append(nop)
                        inst.sync_info = _br.SyncInfo(
                            on_wait=keep, on_update=list(si.on_update or []))
                newl.append(inst)
            if dirty:
                bb.instructions = newl
    return n_split


F = 512; H = 512; C = 100; D = 4
NODES = 15; LEAVES = 16; NBLOCKS = 2
BS = 1024          # per-core batch shard
NT = BS // 128     # b-tiles per core
KT = H // 128      # contraction tiles
EPS = 1e-5
WSCALE = 16.0      # host scale on fp8 head weights / x identity

f32 = mybir.dt.float32
f16 = mybir.dt.float16
f8 = mybir.dt.float8e4
AF = mybir.ActivationFunctionType
ALU = mybir.AluOpType
PM = mybir.MatmulPerfMode

# nodes whose x-term is folded into the fs PSUM on the PE (identity
# matmul); the rest compute the x-dot on DVE. Tunable PE<->DVE balance.
FOLD_NODES = tuple(range(NODES))


def build_nc(skip_blocks):
    nc = bass.Bass("TRN2", target_bir_lowering=False, debug=False, num_devices=1)
    d = {}
    def din(name, shape, dt):
        d[name] = nc.dram_tensor(name, shape, dt, kind="ExternalInput").ap()
    din("xT", [F, BS], f16)
    din("x", [BS, F], f16)
    din("I16", [128, 128], f16)      # 16 * identity
    din("W0", [F, H], f16)
    din("c0", [H, 1], f32)
    if not skip_blocks:
        for i in range(NBLOCKS):
            din(f"W1_{i}", [H, H], f16); din(f"c1_{i}", [H, 1], f32)
            din(f"W2_{i}", [H, H], f16); din(f"c2_{i}", [H, 1], f32)
    din("Wfi", [H, NODES * F], f8)   # 16 * w_fi
    din("Wfs", [H, NODES * F], f8)   # -16 * w_fs
    din("Wlc", [H, LEAVES * C], f16)  # class-major permuted
    y_ap = nc.dram_tensor("y", [BS, C], f32, kind="ExternalOutput").ap()

    with tile.TileContext(nc) as tc, ExitStack() as ctx:
        per = ctx.enter_context(tc.tile_pool(name="per", bufs=1))
        p2ps = ctx.enter_context(tc.tile_pool(name="p2ps", bufs=3, space="PSUM"))
        p3ps = ctx.enter_context(tc.tile_pool(name="p3ps", bufs=2, space="PSUM"))

        xT_w = per.tile([128, KT * BS], f16, name="xT_w")
        _xv = xT_w[:].rearrange("p (k c) -> p k c", k=KT)
        _sv = d["xT"].rearrange("(k p) c -> p k c", k=KT)
        for _k in range(KT):
            nc.sync.dma_start(_xv[:, _k:_k+1], _sv[:, _k:_k+1])
        xT_t = [xT_w[:, k*BS:(k+1)*BS] for k in range(KT)]
        i16 = per.tile([128, 128], f16, name="i16")
        nc.sync.dma_start(i16[:], d["I16"])

        # PE p-state warmup: ~4us of dummy matmuls on memset data while
        # the trunk DMAs land, so the trunk starts at the full 2.4GHz
        # clock instead of ramping through it (saves ~6us of ramp)
        warm = per.tile([128, 512], f16, name="warm")
        nc.gpsimd.memset(warm[:], 0.0)
        NWARM = 20
        wp = p3ps.tile([128, 400], f32, name="lps", tag="lps")
        for i in range(NWARM):
            # one accumulation group: no semaphores between matmuls,
            # so the PE stays continuously busy and ramps its clock
            nc.tensor.matmul(wp[:], warm[:, 0:128], warm[:, 0:400],
                             start=(i == 0), stop=(i == NWARM - 1))

        # ---------------- phase 1: trunk ----------------
        def load_w(pool, name, ap, cols, dt=f16, step=None, eng=None):
            wide = pool.tile([128, KT * cols], dt, name=name, tag=name)
            wv = wide[:].rearrange("p (k c) -> p k c", k=KT)
            sv = ap.rearrange("(k p) c -> p k c", k=KT)
            step = cols if step is None else step
            eng = nc.sync if eng is None else eng
            for k in range(KT):
                for c0 in range(0, cols, step):
                    c1 = min(c0 + step, cols)
                    eng.dma_start(wv[:, k:k+1, c0:c1], sv[:, k:k+1, c0:c1])
            return wide

        def load_c(pool, name, ap):
            # one DMA for all KT bias chunks (sync-engine issues are ~600ns
            # each, so fewer dma_starts matter more than parallel queues)
            tl = pool.tile([128, KT], f32, name=name, tag=name)
            nc.sync.dma_start(tl[:].rearrange("p (k c) -> p k c", c=1),
                              ap.rearrange("(k p) c -> p k c", k=KT))
            return [tl[:, m:m+1] for m in range(KT)]

        def dense_layerT(in_tiles, W_w, c_t, func, out_tiles):
            W_t = [W_w[:, k*H:(k+1)*H] for k in range(KT)]
            for m in range(KT):
                for bc in range(BS // 512):
                    ps = p2ps.tile([128, 512], f32, name="fi_ps", tag="fi_ps")
                    for k in range(KT):
                        nc.tensor.matmul(
                            ps[:], W_t[k][:, m*128:(m+1)*128],
                            in_tiles[k][:, bc*512:(bc+1)*512],
                            start=(k == 0), stop=(k == KT - 1))
                    nc.scalar.activation(
                        out_tiles[m][:, bc*512:(bc+1)*512], ps[:], func,
                        bias=c_t[m][:], scale=1.0)

        hT = [per.tile([128, BS], f16, name=f"hT{m}") for m in range(KT)]
        # fp8 copy of hT, one wide tile so DoubleRow k-pair views work
        hT8 = per.tile([128, KT * BS], f8, name="hT8")
        p1w = ctx.enter_context(tc.tile_pool(name="p1w", bufs=1))
        W0_w = load_w(p1w, "W0t", d["W0"], H)
        c0_t = load_c(p1w, "c0t", d["c0"])
        if skip_blocks:
            # fast path: relu on DVE (bias is zero after BN fold only if
            # c0 is zero - it isn't, so fold bias via scalar_tensor_tensor:
            # out = max(ps + c0, 0) needs bias-add first; keep ACT when a
            # bias exists but emit the fp8 copy per m so the node loop can
            # start as soon as hT k-pair 0 is ready
            W0_t = [W0_w[:, k*H:(k+1)*H] for k in range(KT)]
            for m in range(KT):
                for bc in range(BS // 512):
                    ps = p2ps.tile([128, 512], f32, name="fi_ps", tag="fi_ps")
                    for k in range(KT):
                        nc.tensor.matmul(
                            ps[:], W0_t[k][:, m*128:(m+1)*128],
                            xT_t[k][:, bc*512:(bc+1)*512],
                            start=(k == 0), stop=(k == KT - 1))
                    nc.scalar.activation(
                        hT[m][:, bc*512:(bc+1)*512], ps[:], AF.Relu,
                        bias=c0_t[m][:], scale=1.0)
                    nc.vector.tensor_copy(
                        hT8[:, m*BS+bc*512:m*BS+(bc+1)*512],
                        hT[m][:, bc*512:(bc+1)*512])
        else:
            dense_layerT(xT_t, W0_w, c0_t, AF.Relu, hT)
        if not skip_blocks:
            with tc.tile_pool(name="blkw", bufs=2) as blkw, \
                 tc.tile_pool(name="blk", bufs=1) as blk:
                o1 = [blk.tile([128, BS], f16, name=f"o1_{m}") for m in range(KT)]
                o2 = [blk.tile([128, BS], f16, name=f"o2_{m}") for m in range(KT)]
                o3 = [blk.tile([128, BS], f16, name=f"o3_{m}") for m in range(KT)]
                cur = hT
                for i in range(NBLOCKS):
                    W1_w = load_w(blkw, "Wblk", d[f"W1_{i}"], H)
                    c1_t = load_c(p1w, f"c1t{i}", d[f"c1_{i}"])
                    dense_layerT(cur, W1_w, c1_t, AF.Gelu, o1)
                    W2_w = load_w(blkw, "Wblk", d[f"W2_{i}"], H)
                    c2_t = load_c(p1w, f"c2t{i}", d[f"c2_{i}"])
                    dense_layerT(o1, W2_w, c2_t, AF.Gelu, o2)
                    nxt = hT if cur is o3 else o3
                    for m in range(KT):
                        nc.vector.tensor_add(nxt[m][:], cur[m][:], o2[m][:])
                    cur = nxt
                if cur is not hT:
                    for m in range(KT):
                        nc.vector.tensor_copy(hT[m][:], cur[m][:])
            for m in range(KT):
                nc.vector.tensor_copy(hT8[:, m*BS:(m+1)*BS], hT[m][:])
        hT8v = hT8[:].rearrange("p (k c) -> p k c", k=KT)

        # ---------------- phase 2: fi/fs heads + soft decisions ----------
        x_w = per.tile([128, NT * F], f16, name="x_w")
        _xwv = x_w[:].rearrange("p (t c) -> p t c", t=NT)
        _xsv = d["x"].rearrange("(t p) c -> p t c", t=NT)
        for _t in range(0, NT, 2):
            nc.sync.dma_start(_xwv[:, _t:_t+2], _xsv[:, _t:_t+2])
        x_t = [x_w[:, t*F:(t+1)*F] for t in range(NT)]
        stats = ctx.enter_context(tc.tile_pool(name="stats", bufs=1))
        den_t = [stats.tile([128, NODES], f32, name=f"den{t}") for t in range(NT)]
        num_t = [stats.tile([128, NODES], f32, name=f"num{t}") for t in range(NT)]
        all_fold = len(FOLD_NODES) == NODES
        numx_t = None
        if not all_fold:
            numx_t = [stats.tile([128, NODES], f32, name=f"numx{t}")
                      for t in range(NT)]
            for t in range(NT):
                nc.gpsimd.memset(numx_t[t][:], 0.0)
        lsb_t = [stats.tile([128, LEAVES * C], f16, name=f"lsb{t}")
                 for t in range(NT)]

        hw_pool = ctx.enter_context(tc.tile_pool(name="hw", bufs=1))
        p2sc = ctx.enter_context(tc.tile_pool(name="p2sc", bufs=5))
        lcw = ctx.enter_context(tc.tile_pool(name="lcw", bufs=1))
        small = ctx.enter_context(tc.tile_pool(name="smal", bufs=2))
        p3sc = ctx.enter_context(tc.tile_pool(name="p3sc", bufs=2))

        # all fi/fs head weights resident in SBUF (fp8 halves them to
        # ~61KB/partition total, so no per-node DMA rotation is needed).
        # Issued from the GpSimd queue: the Sync engine's ~600ns-per-DMA
        # serial issue would otherwise gate the whole startup.
        # Interleave the per-node fi/fs loads so node 0 of both arrives first.
        wfi_all_t = hw_pool.tile([128, NODES * KT * F], f8, name="wfi_all",
                                 tag="wfi_all")
        wfs_all_t = hw_pool.tile([128, NODES * KT * F], f8, name="wfs_all",
                                 tag="wfs_all")
        wfi_all = wfi_all_t[:].rearrange("p (n k c) -> p n k c", n=NODES, k=KT)
        wfs_all = wfs_all_t[:].rearrange("p (n k c) -> p n k c", n=NODES, k=KT)
        # Wlc before the bulk head weights; needed from tile 0's lnc on
        wlc_holder = [load_w(lcw, "wlc", d["Wlc"], LEAVES * C, step=800)]
        for n in range(NODES):
            for wv, ap in ((wfi_all, d["Wfi"]), (wfs_all, d["Wfs"])):
                sv = ap[:, n*F:(n+1)*F].rearrange("(k p) c -> p k c", k=KT)
                hk = KT // 2
                nc.sync.dma_start(wv[:, n, 0:hk], sv[:, 0:hk])
                nc.sync.dma_start(wv[:, n, hk:KT], sv[:, hk:KT])

        CC = 4 * C    # 400-col psum chunks (one PSUM bank)
        NCH = (LEAVES * C) // CC

        def emit_lnc(t):
            # leaf logits for tile t -> lsb_t[t] (SBUF, f16), via ACT copies
            wlc_w = wlc_holder[0]
            wlc_t = [wlc_w[:, k*LEAVES*C:(k+1)*LEAVES*C] for k in range(KT)]
            for ci in range(NCH):
                cols = slice(ci * CC, (ci + 1) * CC)
                lps = p3ps.tile([128, CC], f32, name="lps", tag="lps")
                for k in range(KT):
                    nc.tensor.matmul(lps[:], hT[k][:, t*128:(t+1)*128],
                                     wlc_t[k][:, cols],
                                     start=(k == 0), stop=(k == KT-1))
                nc.scalar.copy(lsb_t[t][:, cols], lps[:])

        def sd_chain(t, lo, hi, eng):
            # sigmoid for node cols [lo:hi) -> (sd, nsd) [128, hi-lo] tiles
            if all_fold:
                nf = num_t[t][:, lo:hi]
            else:
                nfw = small.tile([128, hi - lo], f32, name="nf", tag=f"nf{lo}")
                nc.vector.tensor_add(nfw[:], num_t[t][:, lo:hi],
                                     numx_t[t][:, lo:hi])
                nf = nfw[:]
            rden = small.tile([128, hi - lo], f32, name="rden", tag=f"rden{lo}")
            eng.reciprocal(rden[:], den_t[t][:, lo:hi])
            ratio = small.tile([128, hi - lo], f32, name="ratio", tag=f"rat{lo}")
            eng.tensor_tensor(ratio[:], nf, rden[:], op=ALU.mult)
            # sigmoid via Exp (stays on the already-loaded ACT Exp table)
            er = small.tile([128, hi - lo], f32, name="er", tag=f"er{lo}")
            nc.scalar.activation(er[:], ratio[:], AF.Exp, scale=-1.0)
            er1 = small.tile([128, hi - lo], f32, name="er1", tag=f"er1{lo}")
            eng.tensor_scalar(er1[:], er[:], 1.0, None, op0=ALU.add)
            sd = small.tile([128, hi - lo], f32, name="sd", tag=f"sd{lo}")
            eng.reciprocal(sd[:], er1[:])
            nsd = small.tile([128, hi - lo], f32, name="nsd", tag=f"nsd{lo}")
            eng.tensor_scalar(nsd[:], sd[:], -1.0, 1.0,
                              op0=ALU.mult, op1=ALU.add)
            return sd, nsd

        def routing_levels(t, sd, nsd, d_from, d_to, cur, eng, base):
            # build us-levels d_from..d_to from sd/nsd (whose col 0 is
            # global node `base`) and fold them into the running product
            for dlev in range(d_from, d_to + 1):
                w = 1 << (dlev - 1)
                off = w - 1 - base
                u = small.tile([128, 2 * w], f32, name=f"u{dlev}", tag=f"u{dlev}")
                uv = u[:].rearrange("p (a two) -> p a two", two=2)
                eng.tensor_copy(uv[:, :, 0:1], sd[:, off:off+w].unsqueeze(2))
                eng.tensor_copy(uv[:, :, 1:2], nsd[:, off:off+w].unsqueeze(2))
                if cur is None:
                    cur = u
                else:
                    out = small.tile([128, 2 * w], f32, name=f"c{dlev}",
                                     tag=f"c{dlev}")
                    rep = cur[:].unsqueeze(2).broadcast_to([128, w, 2])
                    ov = out[:].rearrange("p (a two) -> p a two", two=2)
                    eng.tensor_mul(ov, rep, uv)
                    cur = out
            return cur

        def finalize_tile(t, eng):
            # eng: Pool while the next tile's node loop overlaps; the last
            # tile (kernel tail) uses DVE with a split critical path
            sd, nsd = sd_chain(t, 0, NODES, eng=nc.vector)
            coeff = routing_levels(t, sd, nsd, 1, D, None, eng, 0)
            # weighted leaf sum: q3 = lsb * coeff (leaf-broadcast), then a
            # pairwise-add tree over the 16 leaf slots (class-major layout:
            # col = c*LEAVES + l, so the reduce is innermost)
            c16 = small.tile([128, LEAVES], f16, name="c16", tag="c16")
            eng.tensor_copy(c16[:], coeff[:])
            halves = ([(eng, 0, C)] if t != NT - 1 else
                      [(nc.vector, 0, 76), (nc.gpsimd, 76, C)])
            leaf_sum(t, c16, halves)

        def leaf_sum(t, c16, halves):
            # weighted leaf sum + per-half output DMA (split across engines
            # for the last tile so the tail chains run concurrently)
            outt = p3sc.tile([128, C], f32, name="outt", tag=f"outt{t%2}")
            for heng, ca, cb in halves:
                nc_ = cb - ca
                q3 = p3sc.tile([128, LEAVES * nc_], f16, name="q3",
                               tag=f"q3{t%2}{ca}")
                q3v = q3[:].rearrange("p (c l) -> p c l", l=LEAVES)
                lv = lsb_t[t][:, ca*LEAVES:cb*LEAVES].rearrange(
                    "p (c l) -> p c l", l=LEAVES)
                cv = c16[:].unsqueeze(1).broadcast_to([128, nc_, LEAVES])
                heng.tensor_mul(q3v, lv, cv)
                tw = LEAVES
                curv = q3v
                while tw > 2:
                    half = tw // 2
                    nxt = p3sc.tile([128, nc_ * half], f16, name=f"tr{half}",
                                    tag=f"tr{half}_{t%2}{ca}")
                    nv = nxt[:].rearrange("p (c l) -> p c l", l=half)
                    heng.tensor_add(nv, curv[:, :, 0:half],
                                    curv[:, :, half:tw])
                    curv = nv
                    tw = half
                heng.tensor_add(
                    outt[:, ca:cb].rearrange("p (c l) -> p c l", l=1),
                    curv[:, :, 0:1], curv[:, :, 1:2])
                nc.sync.dma_start(y_ap[t*128:(t+1)*128, ca:cb], outt[:, ca:cb])

        def node_tile_body(n, t):
            wfiv = wfi_all[:, n]
            wfsv = wfs_all[:, n]
            stat = lambda kp: hT8v[:, 2*kp:2*kp+2, t*128:(t+1)*128]
            fi_ps = p2ps.tile([128, F], f32, name="fi_ps", tag="fi_ps")
            for kp in range(KT // 2):
                nc.tensor.matmul(fi_ps[:], stat(kp), wfiv[:, 2*kp:2*kp+2, :],
                                 start=(kp == 0), stop=(kp == KT//2 - 1),
                                 perf_mode=PM.DoubleRow)
            fs_ps = p2ps.tile([128, F], f32, name="fs_ps", tag="fs_ps")
            fold = n in FOLD_NODES
            if fold:
                # accumulate 16*x into the bank first (16*I identity matmul)
                nc.tensor.matmul(fs_ps[:], i16[:], x_t[t],
                                 start=True, stop=False, skip_group_check=True)
            for kp in range(KT // 2):
                nc.tensor.matmul(fs_ps[:], stat(kp), wfsv[:, 2*kp:2*kp+2, :],
                                 start=(not fold and kp == 0),
                                 stop=(kp == KT//2 - 1),
                                 perf_mode=PM.DoubleRow,
                                 skip_group_check=fold)
            # P = exp(fi); free running sum -> den
            P = p2sc.tile([128, F], f16, name="P", tag="P")
            nc.scalar.activation(P[:], fi_ps[:], AF.Exp, scale=1.0 / WSCALE,
                                 accum_out=den_t[t][:, n:n+1])
            # fs-dot: (fs_ps/16) * P, accum -> num
            #   fold:  fs_ps = 16*(x - fs)  -> num  = sum (x-fs)*P
            #   else:  fs_ps = -16*fs       -> num  = -sum fs*P
            q = p2sc.tile([128, F], f16, name="q", tag="q")
            nc.vector.scalar_tensor_tensor(q[:], fs_ps[:], 1.0 / WSCALE, P[:],
                                           op0=ALU.mult, op1=ALU.mult,
                                           accum_out=num_t[t][:, n:n+1])
            if not fold:
                qx = p2sc.tile([128, F], f16, name="qx", tag="qx")
                nc.vector.scalar_tensor_tensor(qx[:], P[:], 1.0, x_t[t],
                                               op0=ALU.mult, op1=ALU.mult,
                                               accum_out=numx_t[t][:, n:n+1])

        # tile-major single pass: each tile runs all 15 nodes then
        # finalizes; the finalize chain (Pool) hides under the next
        # tile's node loop. Only the last tile's chain is a tail, so it
        # runs on DVE (faster f16 ops) instead.
        for t in range(NT):
            for n in range(NODES):
                node_tile_body(n, t)
                if n == (9 if t == 0 else 2):
                    emit_lnc(t)
            finalize_tile(t, nc.vector if t == NT - 1 else nc.gpsimd)

    return nc


def prep_core_inputs(core, x, w_in, b_in, g0, be0, bw1, bb1, bg1, bbe1,
                     bw2, bb2, bg2, bbe2, w_fi, b_fi, w_fs, b_fs, w_lc, b_lc,
                     skip_blocks):
    """Host-side prep: shard x, fold BN into weights, transpose x."""
    r = 1.0 / np.sqrt(np.float32(1.0) + np.float32(EPS))
    xs = np.ascontiguousarray(x[core*BS:(core+1)*BS]).astype(np.float32)
    f8np = ml_dtypes.float8_e4m3fn
    m = {
        "x": xs.astype(np.float16),
        "xT": np.ascontiguousarray(xs.T).astype(np.float16),
        "I16": (np.eye(128) * WSCALE).astype(np.float16),
        "W0": np.ascontiguousarray(w_in * (g0 * r)[None, :]).astype(np.float16),
        "c0": (b_in * g0 * r + be0).astype(np.float32).reshape(H, 1),
        "Wfi": np.clip(w_fi * WSCALE, -240, 240).astype(f8np),
        "Wfs": np.clip(w_fs * -WSCALE, -240, 240).astype(f8np),
        # class-major leaf-inner layout: col (l*C + c) -> (c*LEAVES + l)
        "Wlc": np.ascontiguousarray(
            w_lc.reshape(H, LEAVES, C).transpose(0, 2, 1).reshape(
                H, LEAVES * C)).astype(np.float16),
    }
    if not skip_blocks:
        for i in range(NBLOCKS):
            s1 = bg1[i] * r
            m[f"W1_{i}"] = np.ascontiguousarray(bw1[i] * s1[None, :]).astype(np.float16)
            m[f"c1_{i}"] = (bb1[i] * s1 + bbe1[i]).astype(np.float32).reshape(H, 1)
            s2 = bg2[i] * r
            m[f"W2_{i}"] = np.ascontiguousarray(bw2[i] * s2[None, :]).astype(np.float16)
            m[f"c2_{i}"] = (bb2[i] * s2 + bbe2[i]).astype(np.float32).reshape(H, 1)
    return m


N_CORES = 8
B_FULL = 8192

# populated by kernel() when BASS_TRACE=1 (NTFF profiling enabled)
last_exec_time_ns = None
last_trace_path = None


def _reference_numpy(x, w_in, b_in, g0, be0, bw1, bb1, bg1, bbe1,
                     bw2, bb2, bg2, bbe2, w_fi, b_fi, w_fs, b_fs,
                     w_lc, b_lc):
    """Pure-numpy fallback for shapes/cases this kernel was not built for."""
    from scipy.special import erf
    def bn(h, g, b):
        return h * (g / np.sqrt(1.0 + EPS)) + b
    def gelu(v):
        return v * 0.5 * (1.0 + erf(v / np.sqrt(2.0)))
    xx = x.astype(np.float64)
    h = np.maximum(bn(xx @ w_in.astype(np.float64) + b_in, g0, be0), 0.0)
    for i in range(bw1.shape[0]):
        r = h
        o = gelu(bn(h @ bw1[i].astype(np.float64) + bb1[i], bg1[i], bbe1[i]))
        o = gelu(bn(o @ bw2[i].astype(np.float64) + bb2[i], bg2[i], bbe2[i]))
        h = o + r
    Bn = xx.shape[0]
    nodes = w_fi.shape[1] // x.shape[1]
    leaves = nodes + 1
    ncls = w_lc.shape[1] // leaves
    fi = (h @ w_fi.astype(np.float64) + b_fi).reshape(Bn, nodes, -1)
    fs = (h @ w_fs.astype(np.float64) + b_fs).reshape(Bn, nodes, -1)
    lnc = (h @ w_lc.astype(np.float64) + b_lc).reshape(Bn, leaves, ncls)
    e = np.exp(fi - fi.max(-1, keepdims=True))
    sfi = e / e.sum(-1, keepdims=True)
    sd = 1.0 / (1.0 + np.exp(-(np.einsum('bnf,bf->bn', sfi, xx)
                               - (sfi * fs).sum(-1))))
    depth = int(np.log2(leaves))
    coeff = np.ones((Bn, leaves))
    for l in range(leaves):
        for dd in range(1, depth + 1):
            node = (2 ** (dd - 1) * (2 ** depth + l) - 2 ** depth) // (2 ** depth)
            side = (l // 2 ** (depth - dd)) % 2
            p = sd[:, node]
            coeff[:, l] *= p if side == 0 else (1.0 - p)
    return np.einsum('bl,blc->bc', coeff, lnc).astype(np.float32)


def kernel(x, w_in, b_in, g0, be0, bw1, bb1, bg1, bbe1, bw2, bb2, bg2, bbe2,
           w_fi, b_fi, w_fs, b_fs, w_lc, b_lc):
    global last_exec_time_ns, last_trace_path
    inputs = dict(x=x, w_in=w_in, b_in=b_in, g0=g0, be0=be0, bw1=bw1,
                  bb1=bb1, bg1=bg1, bbe1=bbe1, bw2=bw2, bb2=bb2, bg2=bg2,
                  bbe2=bbe2, w_fi=w_fi, b_fi=b_fi, w_fs=w_fs, b_fs=b_fs,
                  w_lc=w_lc, b_lc=b_lc)
    inputs = {k: np.asarray(v) for k, v in inputs.items()}
    x = inputs["x"]
    has_bias = (np.any(inputs["b_fi"]) or np.any(inputs["b_fs"])
                or np.any(inputs["b_lc"]))
    if (x.shape != (B_FULL, F) or inputs["w_in"].shape != (F, H)
            or inputs["w_fi"].shape != (H, NODES * F)
            or inputs["w_lc"].shape != (H, LEAVES * C)
            or inputs["bw1"].shape != (NBLOCKS, H, H) or has_bias):
        return _reference_numpy(**inputs)

    # residual blocks are exactly identity when the second BN has zero
    # weight and bias (gelu(0) == 0); detected from the actual values
    skip_blocks = (not np.any(inputs["bg2"])) and (not np.any(inputs["bbe2"]))

    nc = build_nc(skip_blocks)
    fix_sync_waits(nc, max_waits=1)

    in_maps = [prep_core_inputs(c, **inputs, skip_blocks=skip_blocks)
               for c in range(N_CORES)]
    # the axon-tunneled device occasionally reports a transient
    # NRT_EXEC_UNIT_UNRECOVERABLE on execute; retry a couple of times
    import time as _time
    res = None
    for attempt in range(5):
        try:
            res = bass_utils.run_bass_kernel_spmd(
                nc, in_maps, core_ids=list(range(N_CORES)))
            break
        except Exception:
            if attempt == 4:
                raise
            os.environ.setdefault("NEURON_RT_RESET_CORES", "1")
            _time.sleep(12.0)
    last_exec_time_ns = res.exec_time_ns
    last_trace_path = (res.instructions_and_trace[1]
                       if res.instructions_and_trace else None)
    return np.concatenate([res.results[c]["y"] for c in range(N_CORES)],
                          axis=0).astype(np.float32, copy=False)
